# revision 13
# baseline (speedup 1.0000x reference)
"""Level-1 3D Haar DWT on video [4,3,16,256,256] f32 -> 8 subbands
[4,3,8,128,128], pywt convention (cA=(x0+x1)/sqrt2, cD=(x0-x1)/sqrt2 over
frames, height, width).

Distribution: pure data parallel over the 8 frame pairs (F=16 -> 8
independent pairs); core k processes video[:, :, 2k:2k+2] with zero
cross-core communication.

Per-core pipeline (Bass/Tile), ragged chunks of CH pairs, row-half
u in {0,1} (rows u*128..u*128+127 of each frame):
  load (sync HWDGE ring): X[f,u] = x[p, f, u-half]  [128 rows, CH, 256]
  F stage (DVE):  A_u = X[0,u] + X[1,u]; D_u = X[0,u] - X[1,u]
  H stage (PE):   P_t_u = B.T @ (A|D)_u -> PSUM, B (+-1, fp32-exact):
                  out[j] = in[2j] + in[2j+1]      (aa rows 0..63)
                  out[64+j] = in[2j] - in[2j+1]   (ad rows 64..127)
                  TensorE has its own SBUF ports, so this runs fully
                  parallel to the DVE (GpSimd would lock the DVE's
                  second read port instead - measured, not theoretical).
  evac (ACT):     odd columns of P -> SBUF (a 2-input DVE op may read
                  at most one operand from PSUM)
  W stage (DVE):  (xe -/+ xo) * 2^-1.5 via the fused LN_BWD_DX custom
                  op; xe strided from PSUM, xo from SBUF; all three
                  1/sqrt2 stage scales folded here.
  store (scalar HWDGE ring): h-major DRAM layout, 512B+ runs.

Output DRAM y[u, j, t, e, p, w]: subband s = (t, j>=64, e), h = u*64+j%64.
"""

import math

import numpy as np

import concourse.bacc as bacc
import concourse.mybir as mybir
from concourse.bass_utils import run_bass_kernel_spmd
from concourse.tile import TileContext

F32 = mybir.dt.float32
NCORES = 8
NPAIRS = 12
CHUNKS = (2, 4, 4, 2)   # ragged: short first/last chunks trim fill/drain
CHMAX = max(CHUNKS)
NCHUNK = len(CHUNKS)
C3 = (1.0 / math.sqrt(2.0)) ** 3

_CACHE = {}


def _bmat():
    b = np.zeros((128, 128), np.float32)
    for j in range(64):
        b[2 * j, j] = 1.0
        b[2 * j + 1, j] = 1.0
        b[2 * j, 64 + j] = 1.0
        b[2 * j + 1, 64 + j] = -1.0
    return b


def _build_bass():
    nc = bacc.Bacc()
    x = nc.dram_tensor("x", [NPAIRS, 2, 256, 256], F32, kind="ExternalInput")
    bm = nc.dram_tensor("bmat", [128, 128], F32, kind="ExternalInput")
    y = nc.dram_tensor("y", [2, 128, 2, 2, NPAIRS, 128], F32,
                       kind="ExternalOutput")

    add = mybir.AluOpType.add
    sub = mybir.AluOpType.subtract

    with TileContext(nc) as tc:
        with tc.tile_pool(name="const", bufs=1) as cpool, \
             tc.tile_pool(name="io", bufs=3) as io_pool, \
             tc.tile_pool(name="mid", bufs=3) as mid_pool, \
             tc.tile_pool(name="ps", bufs=1, space="PSUM") as ps_pool:
            B = cpool.tile([128, 128], F32, name="B")
            nc.scalar.dma_start(out=B[:, :], in_=bm[:, :])
            p0 = 0
            for ci, CH in enumerate(CHUNKS):
                X = {}
                for u in range(2):
                    for f in range(2):
                        Xt = io_pool.tile([128, CH, 256], F32, name="X",
                                          tag=f"X{f}{u}", bufs=4,
                                          padded_shape=[128, CHMAX, 256])
                        ldeng = nc.sync if ci % 2 == 0 else nc.scalar
                        ldeng.dma_start(
                            out=Xt[:, :, :],
                            in_=x[p0:p0 + CH, f, 128 * u:128 * (u + 1)]
                                .rearrange("p r w -> r p w"),
                        )
                        X[(f, u)] = Xt
                # F stage: A_u = f0 + f1, D_u = f0 - f1
                AD = {}
                for u in range(2):
                    for t in range(2):       # 0: A (sum), 1: D (diff)
                        M = mid_pool.tile([128, CH, 256], F32, name="M",
                                          tag=f"M{t}{u}",
                                          padded_shape=[128, CHMAX, 256])
                        nc.vector.tensor_tensor(
                            M[:, :, :], X[(0, u)][:, :, :], X[(1, u)][:, :, :],
                            add if t == 0 else sub)
                        AD[(t, u)] = M
                # H stage on PE -> PSUM
                E = {}
                for (t, u), M in AD.items():
                    i = 2 * t + u
                    P = ps_pool.tile([128, CH, 256], F32, name="P", tag=f"P{i}",
                                     padded_shape=[128, CHMAX, 256])
                    Pf = P.rearrange("j p w -> j (p w)")
                    Mf = M.rearrange("j p w -> j (p w)")
                    for n0 in range(0, CH * 256, 512):  # one PSUM bank per matmul
                        n1 = min(n0 + 512, CH * 256)
                        nc.tensor.matmul(
                            Pf[:, n0:n1], B[:, :], Mf[:, n0:n1])
                    # evacuate only the odd columns (ACT): the W-stage
                    # 2-input op may read at most one operand from PSUM
                    Od = mid_pool.tile([128, CH * 128], F32, name="Od",
                                       tag=f"O{i}",
                                       padded_shape=[128, CHMAX * 128])
                    nc.scalar.copy(
                        Od[:, :],
                        P.rearrange("j p (w r) -> j (p w) r", r=2)[:, :, 1])
                    E[(t, u)] = (P, Od)
                # W stage (DVE): even cols from PSUM, odd from SBUF, *C3 fused
                for u in range(2):
                    YU = io_pool.tile([128, 2, 2, CH * 128], F32, name="YU",
                                      tag=f"YU{u}",
                                      padded_shape=[128, 2, 2, CHMAX * 128])
                    for t in range(2):
                        P, Od = E[(t, u)]
                        xe = P.rearrange("j p (w r) -> j (p w) r", r=2)[:, :, 0]
                        xo = Od[:, :]
                        # out = (in0 - in1*s0 - s1) * imm2
                        nc.vector.ln_bwd_dx(YU[:, t, 0, :], xe, xo, -1.0, 0.0, C3)
                        nc.vector.ln_bwd_dx(YU[:, t, 1, :], xe, xo, 1.0, 0.0, C3)
                    if ci == len(CHUNKS) - 1:
                        # split the final stores per t: the last DMA is
                        # smaller and issues before the u=1 W ops finish
                        for t in range(2):
                            nc.scalar.dma_start(
                                out=y[u, :, t, :, p0:p0 + CH]
                                    .rearrange("j e p w -> j e (p w)"),
                                in_=YU[:, t],
                            )
                    else:
                        nc.scalar.dma_start(
                            out=y[u, :, :, :, p0:p0 + CH]
                                .rearrange("j t e p w -> j t e (p w)"),
                            in_=YU[:, :, :, :],
                        )
                p0 += CH
    nc.compile()
    return nc


def _get_nc():
    if "nc" not in _CACHE:
        _CACHE["nc"] = _build_bass()
    return _CACHE["nc"]


def _shard_inputs(video):
    video = np.ascontiguousarray(np.asarray(video), dtype=np.float32)
    bm = _bmat()
    in_maps = []
    for k in range(NCORES):
        shard = np.ascontiguousarray(
            video[:, :, 2 * k:2 * k + 2]).reshape(NPAIRS, 2, 256, 256)
        in_maps.append({"x": shard, "bmat": bm})
    return in_maps


def _unshard_outputs(results):
    # y[u, j, t, e, p, w]; j = qq*64 + jj; h = u*64 + jj; s = (t, qq, e)
    ys = np.stack([np.asarray(r["y"]) for r in results])  # [8,2,128,2,2,12,128]
    ys = ys.reshape(NCORES, 2, 2, 64, 2, 2, NPAIRS, 128)
    #      dims: (f, u, qq, jj, t, e, p, w)
    ys = ys.transpose(4, 2, 5, 6, 0, 1, 3, 7)
    #      -> (t, qq, e, p, f, u, jj, w)
    ys = ys.reshape(8, 4, 3, NCORES, 128, 128)            # (s, b, c, f, h, w)
    return tuple(np.ascontiguousarray(ys[s]) for s in range(8))


def run(video, **spmd_kwargs):
    nc = _get_nc()
    res = run_bass_kernel_spmd(
        nc, _shard_inputs(video), core_ids=list(range(NCORES)), **spmd_kwargs
    )
    return _unshard_outputs(res.results), res


def kernel(video):
    out, _ = run(video)
    return out


# revision 14
# speedup vs baseline: 1.0897x; 1.0897x over previous
"""Level-1 3D Haar DWT on video [4,3,16,256,256] f32 -> 8 subbands
[4,3,8,128,128], pywt convention (cA=(x0+x1)/sqrt2, cD=(x0-x1)/sqrt2 over
frames, height, width).

Distribution: pure data parallel over the 8 frame pairs (F=16 -> 8
independent pairs); core k processes video[:, :, 2k:2k+2] with zero
cross-core communication.

Per-core pipeline (Bass/Tile), ragged chunks of CH pairs, row-half
u in {0,1} (rows u*128..u*128+127 of each frame):
  load (sync HWDGE ring): X[f,u] = x[p, f, u-half]  [128 rows, CH, 256]
  F stage (DVE):  A_u = X[0,u] + X[1,u]; D_u = X[0,u] - X[1,u]
  H stage (PE):   P_t_u = B.T @ (A|D)_u -> PSUM, B (+-1, fp32-exact):
                  out[j] = in[2j] + in[2j+1]      (aa rows 0..63)
                  out[64+j] = in[2j] - in[2j+1]   (ad rows 64..127)
                  TensorE has its own SBUF ports, so this runs fully
                  parallel to the DVE (GpSimd would lock the DVE's
                  second read port instead - measured, not theoretical).
  evac (ACT):     odd columns of P -> SBUF (a 2-input DVE op may read
                  at most one operand from PSUM)
  W stage (DVE):  (xe -/+ xo) * 2^-1.5 via the fused LN_BWD_DX custom
                  op; xe strided from PSUM, xo from SBUF; all three
                  1/sqrt2 stage scales folded here.
  store (scalar HWDGE ring): h-major DRAM layout, 512B+ runs.

Output DRAM y[u, j, t, e, p, w]: subband s = (t, j>=64, e), h = u*64+j%64.
"""

import math

import numpy as np

import concourse.bacc as bacc
import concourse.mybir as mybir
from concourse.bass_utils import run_bass_kernel_spmd
from concourse.tile import TileContext

F32 = mybir.dt.float32
NCORES = 8
NPAIRS = 12
CHUNKS = (2, 4, 4, 2)   # ragged: short first/last chunks trim fill/drain
CHMAX = max(CHUNKS)
NCHUNK = len(CHUNKS)
C3 = (1.0 / math.sqrt(2.0)) ** 3

_CACHE = {}


def _bmat():
    b = np.zeros((128, 128), np.float32)
    for j in range(64):
        b[2 * j, j] = 1.0
        b[2 * j + 1, j] = 1.0
        b[2 * j, 64 + j] = 1.0
        b[2 * j + 1, 64 + j] = -1.0
    return b


def _build_bass():
    nc = bacc.Bacc()
    x = nc.dram_tensor("x", [NPAIRS, 2, 256, 256], F32, kind="ExternalInput")
    bm = nc.dram_tensor("bmat", [128, 128], F32, kind="ExternalInput")
    y = nc.dram_tensor("y", [2, 128, 2, 2, NPAIRS, 128], F32,
                       kind="ExternalOutput")

    add = mybir.AluOpType.add
    sub = mybir.AluOpType.subtract

    with TileContext(nc) as tc:
        with tc.tile_pool(name="const", bufs=1) as cpool, \
             tc.tile_pool(name="io", bufs=3) as io_pool, \
             tc.tile_pool(name="mid", bufs=3) as mid_pool, \
             tc.tile_pool(name="ps", bufs=1, space="PSUM") as ps_pool:
            B = cpool.tile([128, 128], F32, name="B")
            nc.scalar.dma_start(out=B[:, :], in_=bm[:, :])
            p0 = 0
            for ci, CH in enumerate(CHUNKS):
                X = {}
                for u in range(2):
                    for f in range(2):
                        Xt = io_pool.tile([128, CH, 256], F32, name="X",
                                          tag=f"X{f}{u}", bufs=4,
                                          padded_shape=[128, CHMAX, 256])
                        nc.sync.dma_start(
                            out=Xt[:, :, :],
                            in_=x[p0:p0 + CH, f, 128 * u:128 * (u + 1)]
                                .rearrange("p r w -> r p w"),
                        )
                        X[(f, u)] = Xt
                # F stage: A_u = f0 + f1, D_u = f0 - f1
                AD = {}
                for u in range(2):
                    for t in range(2):       # 0: A (sum), 1: D (diff)
                        M = mid_pool.tile([128, CH, 256], F32, name="M",
                                          tag=f"M{t}{u}",
                                          padded_shape=[128, CHMAX, 256])
                        nc.vector.tensor_tensor(
                            M[:, :, :], X[(0, u)][:, :, :], X[(1, u)][:, :, :],
                            add if t == 0 else sub)
                        AD[(t, u)] = M
                # H stage on PE -> PSUM
                E = {}
                for (t, u), M in AD.items():
                    i = 2 * t + u
                    P = ps_pool.tile([128, CH, 256], F32, name="P", tag=f"P{i}",
                                     padded_shape=[128, CHMAX, 256])
                    Pf = P.rearrange("j p w -> j (p w)")
                    Mf = M.rearrange("j p w -> j (p w)")
                    for n0 in range(0, CH * 256, 512):  # one PSUM bank per matmul
                        n1 = min(n0 + 512, CH * 256)
                        nc.tensor.matmul(
                            Pf[:, n0:n1], B[:, :], Mf[:, n0:n1])
                    # evacuate only the odd columns (ACT): the W-stage
                    # 2-input op may read at most one operand from PSUM
                    Od = mid_pool.tile([128, CH * 128], F32, name="Od",
                                       tag=f"O{i}",
                                       padded_shape=[128, CHMAX * 128])
                    nc.scalar.copy(
                        Od[:, :],
                        P.rearrange("j p (w r) -> j (p w) r", r=2)[:, :, 1])
                    E[(t, u)] = (P, Od)
                # W stage (DVE): even cols from PSUM, odd from SBUF, *C3 fused
                for u in range(2):
                    YU = io_pool.tile([128, 2, 2, CH * 128], F32, name="YU",
                                      tag=f"YU{u}",
                                      padded_shape=[128, 2, 2, CHMAX * 128])
                    for t in range(2):
                        P, Od = E[(t, u)]
                        xe = P.rearrange("j p (w r) -> j (p w) r", r=2)[:, :, 0]
                        xo = Od[:, :]
                        # out = (in0 - in1*s0 - s1) * imm2
                        nc.vector.ln_bwd_dx(YU[:, t, 0, :], xe, xo, -1.0, 0.0, C3)
                        nc.vector.ln_bwd_dx(YU[:, t, 1, :], xe, xo, 1.0, 0.0, C3)
                    if ci == len(CHUNKS) - 1:
                        # split the final stores per t: the last DMA is
                        # smaller and issues before the u=1 W ops finish
                        for t in range(2):
                            nc.scalar.dma_start(
                                out=y[u, :, t, :, p0:p0 + CH]
                                    .rearrange("j e p w -> j e (p w)"),
                                in_=YU[:, t],
                            )
                    else:
                        nc.scalar.dma_start(
                            out=y[u, :, :, :, p0:p0 + CH]
                                .rearrange("j t e p w -> j t e (p w)"),
                            in_=YU[:, :, :, :],
                        )
                p0 += CH
    nc.compile()
    return nc


def _get_nc():
    if "nc" not in _CACHE:
        _CACHE["nc"] = _build_bass()
    return _CACHE["nc"]


def _shard_inputs(video):
    video = np.ascontiguousarray(np.asarray(video), dtype=np.float32)
    bm = _bmat()
    in_maps = []
    for k in range(NCORES):
        shard = np.ascontiguousarray(
            video[:, :, 2 * k:2 * k + 2]).reshape(NPAIRS, 2, 256, 256)
        in_maps.append({"x": shard, "bmat": bm})
    return in_maps


def _unshard_outputs(results):
    # y[u, j, t, e, p, w]; j = qq*64 + jj; h = u*64 + jj; s = (t, qq, e)
    ys = np.stack([np.asarray(r["y"]) for r in results])  # [8,2,128,2,2,12,128]
    ys = ys.reshape(NCORES, 2, 2, 64, 2, 2, NPAIRS, 128)
    #      dims: (f, u, qq, jj, t, e, p, w)
    ys = ys.transpose(4, 2, 5, 6, 0, 1, 3, 7)
    #      -> (t, qq, e, p, f, u, jj, w)
    ys = ys.reshape(8, 4, 3, NCORES, 128, 128)            # (s, b, c, f, h, w)
    return tuple(np.ascontiguousarray(ys[s]) for s in range(8))


def run(video, **spmd_kwargs):
    nc = _get_nc()
    res = run_bass_kernel_spmd(
        nc, _shard_inputs(video), core_ids=list(range(NCORES)), **spmd_kwargs
    )
    return _unshard_outputs(res.results), res


def kernel(video):
    out, _ = run(video)
    return out


# revision 15
# speedup vs baseline: 1.1002x; 1.0096x over previous
"""Level-1 3D Haar DWT on video [4,3,16,256,256] f32 -> 8 subbands
[4,3,8,128,128], pywt convention (cA=(x0+x1)/sqrt2, cD=(x0-x1)/sqrt2 over
frames, height, width).

Distribution: pure data parallel over the 8 frame pairs (F=16 -> 8
independent pairs); core k processes video[:, :, 2k:2k+2] with zero
cross-core communication.

Per-core pipeline (Bass/Tile), ragged chunks of CH pairs, row-half
u in {0,1} (rows u*128..u*128+127 of each frame):
  load (sync HWDGE ring): X[f,u] = x[p, f, u-half]  [128 rows, CH, 256]
  F stage (DVE):  A_u = X[0,u] + X[1,u]; D_u = X[0,u] - X[1,u]
  H stage (PE):   P_t_u = B.T @ (A|D)_u -> PSUM, B (+-1, fp32-exact):
                  out[j] = in[2j] + in[2j+1]      (aa rows 0..63)
                  out[64+j] = in[2j] - in[2j+1]   (ad rows 64..127)
                  TensorE has its own SBUF ports, so this runs fully
                  parallel to the DVE (GpSimd would lock the DVE's
                  second read port instead - measured, not theoretical).
  evac (ACT):     odd columns of P -> SBUF (a 2-input DVE op may read
                  at most one operand from PSUM)
  W stage (DVE):  (xe -/+ xo) * 2^-1.5 via the fused LN_BWD_DX custom
                  op; xe strided from PSUM, xo from SBUF; all three
                  1/sqrt2 stage scales folded here.
  store (scalar HWDGE ring): h-major DRAM layout, 512B+ runs.

Output DRAM y[u, j, t, e, p, w]: subband s = (t, j>=64, e), h = u*64+j%64.
"""

import math

import numpy as np

import concourse.bacc as bacc
import concourse.mybir as mybir
from concourse.bass_utils import run_bass_kernel_spmd
from concourse.tile import TileContext

F32 = mybir.dt.float32
NCORES = 8
NPAIRS = 12
CHUNKS = (2, 4, 4, 2)   # ragged: short first/last chunks trim fill/drain
CHMAX = max(CHUNKS)
NCHUNK = len(CHUNKS)
C3 = (1.0 / math.sqrt(2.0)) ** 3

_CACHE = {}


def _bmat():
    b = np.zeros((128, 128), np.float32)
    for j in range(64):
        b[2 * j, j] = 1.0
        b[2 * j + 1, j] = 1.0
        b[2 * j, 64 + j] = 1.0
        b[2 * j + 1, 64 + j] = -1.0
    return b


def _build_bass():
    nc = bacc.Bacc()
    x = nc.dram_tensor("x", [NPAIRS, 2, 256, 256], F32, kind="ExternalInput")
    bm = nc.dram_tensor("bmat", [128, 128], F32, kind="ExternalInput")
    y = nc.dram_tensor("y", [2, 128, 2, 2, NPAIRS, 128], F32,
                       kind="ExternalOutput")

    add = mybir.AluOpType.add
    sub = mybir.AluOpType.subtract

    with TileContext(nc) as tc:
        with tc.tile_pool(name="const", bufs=1) as cpool, \
             tc.tile_pool(name="io", bufs=3) as io_pool, \
             tc.tile_pool(name="mid", bufs=3) as mid_pool, \
             tc.tile_pool(name="ps", bufs=1, space="PSUM") as ps_pool:
            B = cpool.tile([128, 128], F32, name="B")
            nc.scalar.dma_start(out=B[:, :], in_=bm[:, :])
            p0 = 0
            for ci, CH in enumerate(CHUNKS):
                X = {}
                for u in range(2):
                    for f in range(2):
                        Xt = io_pool.tile([128, CH, 256], F32, name="X",
                                          tag=f"X{f}{u}", bufs=4,
                                          padded_shape=[128, CHMAX, 256])
                        nc.sync.dma_start(
                            out=Xt[:, :, :],
                            in_=x[p0:p0 + CH, f, 128 * u:128 * (u + 1)]
                                .rearrange("p r w -> r p w"),
                        )
                        X[(f, u)] = Xt
                # F stage: A_u = f0 + f1, D_u = f0 - f1
                AD = {}
                for u in range(2):
                    for t in range(2):       # 0: A (sum), 1: D (diff)
                        M = mid_pool.tile([128, CH, 256], F32, name="M",
                                          tag=f"M{t}{u}",
                                          padded_shape=[128, CHMAX, 256])
                        nc.vector.tensor_tensor(
                            M[:, :, :], X[(0, u)][:, :, :], X[(1, u)][:, :, :],
                            add if t == 0 else sub)
                        AD[(t, u)] = M
                # H stage on PE -> PSUM
                E = {}
                for (t, u), M in AD.items():
                    i = 2 * t + u
                    P = ps_pool.tile([128, CH, 256], F32, name="P", tag=f"P{i}",
                                     padded_shape=[128, CHMAX, 256])
                    Pf = P.rearrange("j p w -> j (p w)")
                    Mf = M.rearrange("j p w -> j (p w)")
                    for n0 in range(0, CH * 256, 512):  # one PSUM bank per matmul
                        n1 = min(n0 + 512, CH * 256)
                        nc.tensor.matmul(
                            Pf[:, n0:n1], B[:, :], Mf[:, n0:n1])
                    # evacuate only the odd columns (ACT): the W-stage
                    # 2-input op may read at most one operand from PSUM
                    Od = mid_pool.tile([128, CH * 128], F32, name="Od",
                                       tag=f"O{i}",
                                       padded_shape=[128, CHMAX * 128])
                    nc.scalar.copy(
                        Od[:, :],
                        P.rearrange("j p (w r) -> j (p w) r", r=2)[:, :, 1])
                    E[(t, u)] = (P, Od)
                # W stage (DVE): even cols from PSUM, odd from SBUF, *C3 fused
                for u in range(2):
                    YU = io_pool.tile([128, 2, 2, CH * 128], F32, name="YU",
                                      tag=f"YU{u}",
                                      padded_shape=[128, 2, 2, CHMAX * 128])
                    for t in range(2):
                        P, Od = E[(t, u)]
                        xe = P.rearrange("j p (w r) -> j (p w) r", r=2)[:, :, 0]
                        xo = Od[:, :]
                        # out = (in0 - in1*s0 - s1) * imm2
                        nc.vector.ln_bwd_dx(YU[:, t, 0, :], xe, xo, -1.0, 0.0, C3)
                        nc.vector.ln_bwd_dx(YU[:, t, 1, :], xe, xo, 1.0, 0.0, C3)
                    nc.scalar.dma_start(
                        out=y[u, :, :, :, p0:p0 + CH]
                            .rearrange("j t e p w -> j t e (p w)"),
                        in_=YU[:, :, :, :],
                    )
                p0 += CH
    nc.compile()
    return nc


def _get_nc():
    if "nc" not in _CACHE:
        _CACHE["nc"] = _build_bass()
    return _CACHE["nc"]


def _shard_inputs(video):
    video = np.ascontiguousarray(np.asarray(video), dtype=np.float32)
    bm = _bmat()
    in_maps = []
    for k in range(NCORES):
        shard = np.ascontiguousarray(
            video[:, :, 2 * k:2 * k + 2]).reshape(NPAIRS, 2, 256, 256)
        in_maps.append({"x": shard, "bmat": bm})
    return in_maps


def _unshard_outputs(results):
    # y[u, j, t, e, p, w]; j = qq*64 + jj; h = u*64 + jj; s = (t, qq, e)
    ys = np.stack([np.asarray(r["y"]) for r in results])  # [8,2,128,2,2,12,128]
    ys = ys.reshape(NCORES, 2, 2, 64, 2, 2, NPAIRS, 128)
    #      dims: (f, u, qq, jj, t, e, p, w)
    ys = ys.transpose(4, 2, 5, 6, 0, 1, 3, 7)
    #      -> (t, qq, e, p, f, u, jj, w)
    ys = ys.reshape(8, 4, 3, NCORES, 128, 128)            # (s, b, c, f, h, w)
    return tuple(np.ascontiguousarray(ys[s]) for s in range(8))


def run(video, **spmd_kwargs):
    nc = _get_nc()
    res = run_bass_kernel_spmd(
        nc, _shard_inputs(video), core_ids=list(range(NCORES)), **spmd_kwargs
    )
    return _unshard_outputs(res.results), res


def kernel(video):
    out, _ = run(video)
    return out
